# revision 53
# baseline (speedup 1.0000x reference)
"""Causal multi-head attention (B=2, H=16, S=2048, D=64, fp32 I/O) on 8 TRN2
NeuronCores.

Sharding: batch*heads (32 units) split 4-per-core — embarrassingly parallel,
no collectives.

Per-core kernel design (bf16 compute, fp32 PSUM accumulation):
  - scores are computed TRANSPOSED: scoresT[k, q] = K_blk @ Q^T so that the
    softmax numerators P^T[k, q] feed the P@V matmul directly as the moving
    operand (contraction dim k on partitions), with V (natural layout) as the
    stationary operand.
  - A ones-column appended to V accumulates the softmax denominator l[q] in
    the same PSUM accumulation as P@V — no separate reduction pass.
  - exp is fused with the PSUM->SBUF eviction on ScalarE (scale=1/sqrt(D)
    folded into the activation). No max-subtraction: scores ~ N(0,1), no
    overflow risk in fp32 exp.
  - Causal masking: off-diagonal blocks are skipped entirely; diagonal blocks
    get a multiplicative 0/1 upper-triangular mask after exp.
  - q is processed in halves of 1024 so PSUM fits: out^T[65,1024] (2 banks,
    double-buffered) + scoresT slots [128,1024] (2 banks, double-buffered).
  - Epilogue: copy out^T to SBUF bf16, DMA-transpose back to natural layout
    (the l column rides along), reciprocal + per-partition scale, f32 out.
  - Q^T/K^T layouts are built with bf16 DMA-transposes; K^T lands directly in
    a "paired slab" layout (kj even on partitions 0:64, kj odd on 64:128) so
    consecutive kj matmuls use disjoint PE row-groups and overlap in-array.
"""

import numpy as np

import concourse.bass as bass
import concourse.mybir as mybir
import concourse.tile as tile
from concourse.tile import add_dep_helper
from concourse import bacc
from concourse.bass_utils import run_bass_kernel_spmd
from concourse.masks import make_upper_triangular

B, H, S, D = 2, 16, 2048, 64
N_CORES = 8
HPC = (B * H) // N_CORES  # heads per core
NT = S // 128  # 16 k/q blocks of 128
FP32 = mybir.dt.float32
BF16 = mybir.dt.bfloat16


def build_attention():
    nc = bacc.Bacc("TRN2", target_bir_lowering=False)
    q_d = nc.dram_tensor("query", [HPC, S, D], FP32, kind="ExternalInput")
    k_d = nc.dram_tensor("key", [HPC, S, D], FP32, kind="ExternalInput")
    v_d = nc.dram_tensor("value", [HPC, S, D], FP32, kind="ExternalInput")
    o_d = nc.dram_tensor("out", [HPC, S, D], FP32, kind="ExternalOutput")

    with tile.TileContext(nc) as tc:
        with (
            tc.tile_pool(name="singles", bufs=1) as singles,
            tc.tile_pool(name="nat", bufs=4) as nat_pool,
            tc.tile_pool(name="bf", bufs=3) as bf_pool,
            tc.tile_pool(name="slab", bufs=2) as slab_pool,
            tc.tile_pool(name="qt", bufs=2) as qt_pool,
            tc.tile_pool(name="pt", bufs=7) as pt_pool,
            tc.tile_pool(name="ep", bufs=4) as ep_pool,
            tc.tile_pool(name="sc", bufs=2, space="PSUM") as sc_pool,
            tc.tile_pool(name="ops", bufs=2, space="PSUM") as ops_pool,
        ):
            # 0/1 mask, keep k <= q (partition = k, free = q)
            tri01 = singles.tile([128, 128], BF16, tag="tri01")
            make_upper_triangular(nc, tri01, val=1.0, diag=True)

            hm = NT // 2
            # first piece small: shortens the load->cast->transpose chain
            # in front of the very first matmul
            PIECES = ((0, 8), (8, 16))

            def load_head(h, pieces=PIECES):
                """DMA the head's q/k to SBUF (issued early; no DVE work
                so nothing blocks the DVE queue on in-flight load data).
                V is loaded separately, after the transposes: the PV
                matmuls lag a pair behind, so V is off the critical path
                and its in-flight data would otherwise delay them."""
                natQ = nat_pool.tile([128, NT, D], FP32, tag="natQ", name="natQ")
                natK = nat_pool.tile([128, NT, D], FP32, tag="natK", name="natK")
                qsrc = q_d[h].rearrange("(t p) d -> p t d", p=128)
                ksrc = k_d[h].rearrange("(t p) d -> p t d", p=128)
                for a, b in pieces:
                    nc.sync.dma_start(out=natQ[:, a:b, :], in_=qsrc[:, a:b, :])
                    nc.sync.dma_start(out=natK[:, a:b, :], in_=ksrc[:, a:b, :])
                return natQ, natK

            def load_v(h):
                natV = nat_pool.tile([128, NT, D], FP32, tag="natV", name="natV")
                vsrc = v_d[h].rearrange("(t p) d -> p t d", p=128)
                nc.sync.dma_start(out=natV[:, 0:hm, :], in_=vsrc[:, 0:hm, :])
                nc.sync.dma_start(out=natV[:, hm:NT, :], in_=vsrc[:, hm:NT, :])
                return natV

            def cast_v(natV, after=()):
                vaug = bf_pool.tile(
                    [128, NT, D + 1], BF16, tag="vaug", name="vaug"
                )
                c = nc.vector.tensor_copy(vaug[:, :, 0:D], natV)
                nc.vector.memset(vaug[:, :, D : D + 1], 1.0)
                for a in after:
                    add_dep_helper(c.ins, a.ins, sync=False,
                                   reason="v cast after epilogue copies")
                return vaug

            def cast_head(nat, after=(), pieces=PIECES):
                """bf16 casts (bfQ2 duplicates each 64-col d-block so its
                128-wide transposed chunks land Q^T_j on BOTH partition
                halves)."""
                natQ, natK = nat
                bfQ2 = bf_pool.tile(
                    [128, NT, 2, D], BF16, tag="bfQ2", name="bfQ2"
                )
                bfK = bf_pool.tile([128, NT, D], BF16, tag="bfK", name="bfK")
                casts = []
                for sl in (slice(a, b) for a, b in pieces):
                    casts.append(nc.vector.tensor_copy(bfQ2[:, sl, 0, :], natQ[:, sl, :]))
                    casts.append(nc.vector.tensor_copy(bfQ2[:, sl, 1, :], natQ[:, sl, :]))
                    casts.append(nc.vector.tensor_copy(bfK[:, sl, :], natK[:, sl, :]))
                for c in casts:
                    for a in after:
                        add_dep_helper(c.ins, a.ins, sync=False,
                                       reason="casts after epilogue copies")
                return bfQ2, bfK

            def transpose_setup(bfQ2, bfK, pieces=PIECES):
                """Blocked DMA-transposes: kslab pairs (K^T_{2j} on rows 0:64,
                K^T_{2j+1} on rows 64:128) and q-contiguous qt."""
                kslab = slab_pool.tile(
                    [128, NT // 2, 128], BF16, tag="kslab", name="kslab"
                )
                qt3 = qt_pool.tile([128, NT, 128], BF16, tag="qt", name="qt")
                bfK_f = bfK.rearrange("p t d -> p (t d)")
                bfQ2_f = bfQ2.rearrange("p t c d -> p (t c d)")
                for j0, j1 in ((a // 2, b // 2) for a, b in pieces):
                    nc.sync.dma_start_transpose(
                        out=kslab[:, j0:j1, :],
                        in_=bfK_f[:, j0 * 128 : j1 * 128],
                    )
                    nc.sync.dma_start_transpose(
                        out=qt3[:, 2 * j0 : 2 * j1, :],
                        in_=bfQ2_f[:, j0 * 256 : j1 * 256],
                    )
                return kslab, qt3.rearrange("p t i -> p (t i)")

            def cast_head_fused(nat, after=()):
                """bf16 casts into ONE combined per-half buffer
                [K-half (512) | Q2-half (1024)] so each half needs a single
                blocked DMA-transpose (fewer xbar serialization points)."""
                natQ, natK = nat
                # bfkq[:, ph, 0:512] = K tiles of half ph; [:, ph, 512:1536]
                # = duplicated Q chunks of half ph
                bfkq = bf_pool.tile(
                    [128, 2, 1536], BF16, tag="bfkq", name="bfkq"
                )
                casts = []
                for ph in range(2):
                    sl = slice(ph * hm, (ph + 1) * hm)
                    kv = bfkq[:, ph, 0:512].rearrange("p (t d) -> p t d", d=D)
                    qv = bfkq[:, ph, 512:1536].rearrange(
                        "p (t c d) -> p t c d", c=2, d=D
                    )
                    casts.append(nc.vector.tensor_copy(kv, natK[:, sl, :]))
                    casts.append(nc.vector.tensor_copy(qv[:, :, 0, :], natQ[:, sl, :]))
                    casts.append(nc.vector.tensor_copy(qv[:, :, 1, :], natQ[:, sl, :]))
                for c in casts:
                    for a in after:
                        add_dep_helper(c.ins, a.ins, sync=False,
                                       reason="casts after epilogue copies")
                return bfkq

            def transpose_setup_fused(bfkq):
                """One blocked DMA-transpose per half. Output blocks of half
                ph: 0-3 = kslab pairs 4ph..4ph+3, 4-11 = qt tiles 8ph..8ph+7
                (qt region is contiguous within each half, which is all the
                rhs slices ever span)."""
                trs = slab_pool.tile(
                    [128, 2, 12, 128], BF16, tag="trs", name="trs"
                )
                for ph in range(2):
                    nc.sync.dma_start_transpose(
                        out=trs[:, ph, :, :],
                        in_=bfkq[:, ph, :],
                    )
                trs_f = trs.rearrange("p a b c -> p (a b c)")

                def kslab_ap(rows, j):
                    return trs[rows : rows + 64, j // 4, j % 4, :]

                def qt_ap(rows, ca, cb):
                    hf = ca // 1024
                    base = hf * 1536 + 512
                    return trs_f[
                        rows : rows + 64,
                        base + ca - 1024 * hf : base + cb - 1024 * hf,
                    ]

                return kslab_ap, qt_ap

            # software-prefetch pipeline over heads: head h+1's LOADS are
            # emitted before head h's compute (the Sync engine issues its
            # stream in order), but its CASTS between the two half-sections
            # so they queue on DVE after half 0's epilogue copy (which gates
            # the single-buffered PSUM accumulator).
            staged = cast_head(load_head(0))
            nat_next = None
            natv_next = None
            vaug = None
            last_ep_copies = []
            for h in range(HPC):
                bfQ2, bfK = staged
                kslab, qt = transpose_setup(bfQ2, bfK)
                if h == 0:
                    vaug = cast_v(load_v(0))
                if h + 1 < HPC:
                    nat_next = load_head(h + 1)
                    natv_next = load_v(h + 1)

                # ---- main loop: q halves x k blocks ----
                for hf in range(2):
                    if hf == 1 and h + 1 < HPC:
                        staged = cast_head(nat_next, after=tuple(last_ep_copies))
                        vaug_next = cast_v(natv_next, after=tuple(last_ep_copies))
                    q0 = 1024 * hf  # absolute start of this q-half
                    q1 = q0 + 1024
                    kj_hi = 8 * (hf + 1)  # kj in [0, kj_hi)
                    # last kj writing each 512-bank of out^T (for stop flags)
                    last_kj = [
                        max(
                            kj
                            for kj in range(kj_hi)
                            if max(q0, 128 * kj) < q0 + 512 * (b + 1)
                        )
                        for b in range(2)
                    ]

                    outps = ops_pool.tile([80, 2, 512], FP32, tag="outps")
                    outps_f = outps.rearrange("p a b -> p (a b)")

                    # kj processed in even/odd pairs: the two QK^T matmuls use
                    # disjoint PE row-groups (partitions 0:64 vs 64:128) and
                    # run concurrently in-array. The PV matmuls are
                    # software-pipelined one pair behind: the PE queue is
                    # strictly in-order, so a PV issued right after its exp
                    # would head-of-line-block the next pair's independent
                    # QK matmuls while waiting on ScalarE.
                    def emit_pv(pair, qas, chunks):
                        # lane-outer: one V_aug weight load per kj; matmuls
                        # split on the absolute 512 grid (PSUM bank limit)
                        for lane, (kj, qa) in enumerate(zip(pair, qas)):
                            for ca, cb, ptile in chunks:
                                lo = max(ca, qa)
                                while lo < cb:
                                    hi = min(cb, q0 + 512 * ((lo - q0) // 512 + 1))
                                    b = (lo - q0) // 512
                                    nc.tensor.matmul(
                                        outps_f[0:65, lo - q0 : hi - q0],
                                        vaug[:, kj, :],
                                        ptile[:, lane, lo - ca : hi - ca],
                                        start=(kj == 0),
                                        stop=(kj == last_kj[b]),
                                    )
                                    lo = hi

                    pending = []
                    for pj in range(kj_hi // 2):
                        pair = (2 * pj, 2 * pj + 1)
                        qas = [max(q0, 128 * kj) for kj in pair]
                        # Both lanes of a pair share one PSUM slot and one exp:
                        # the two QK^T matmuls then become ready together (same
                        # WAR release) and execute concurrently in disjoint PE
                        # row-groups.
                        chunks = []
                        for ca in range(qas[0], q1, 512):
                            cb = min(ca + 512, q1)
                            cols = cb - ca
                            slot = sc_pool.tile(
                                [128, 2, 512], FP32, tag="slot", name="slot"
                            )
                            for lane, (kj, qa) in enumerate(zip(pair, qas)):
                                lo = max(ca, qa)
                                if lo >= cb:
                                    continue
                                rows = (kj % 2) * 64
                                nc.tensor.matmul(
                                    slot[:, lane, lo - ca : cols],
                                    kslab[rows : rows + 64, kj // 2, :],
                                    qt[rows : rows + 64, lo:cb],
                                    start=True,
                                    stop=True,
                                )
                            ptile = pt_pool.tile(
                                [128, 2, 512], BF16, tag="ptile", name="ptile"
                            )
                            # the odd lane's first 128 cols in its diagonal
                            # chunk exp stale PSUM; PV never reads them
                            nc.scalar.activation(
                                ptile[:, :, 0:cols],
                                slot[:, :, 0:cols],
                                mybir.ActivationFunctionType.Exp,
                                scale=1.0 / np.sqrt(D),
                            )
                            for lane, (kj, qa) in enumerate(zip(pair, qas)):
                                dg = 128 * kj
                                if ca <= dg < cb:  # diagonal block in chunk
                                    # on gpsimd: keeps the mask off the DVE
                                    # queue, which cast/epilogue work clogs
                                    # at head boundaries
                                    nc.gpsimd.tensor_mul(
                                        ptile[:, lane, dg - ca : dg - ca + 128],
                                        ptile[:, lane, dg - ca : dg - ca + 128],
                                        tri01,
                                    )
                            chunks.append((ca, cb, ptile))
                        pending.append((pair, qas, chunks))
                        if pj >= 1:
                            emit_pv(*pending.pop(0))
                    for args in pending:
                        emit_pv(*args)

                    # ---- epilogue for this (head, half) ----
                    # rows 65:80 copy PSUM garbage; they transpose into
                    # columns 65:80 of onat which are never read
                    bfo = ep_pool.tile([80, 1024], BF16, tag="bfo")
                    # per-bank copies: each releases its PSUM bank (WAR) as
                    # soon as it finishes, unblocking the next half's PV
                    epc1 = nc.vector.tensor_copy(bfo[:, 0:512], outps_f[0:80, 0:512])
                    epc2 = nc.vector.tensor_copy(bfo[:, 512:1024], outps_f[0:80, 512:1024])
                    last_ep_copies[:] = [epc1, epc2]
                    onat = ep_pool.tile([128, 8, 80], BF16, tag="onat")
                    nc.sync.dma_start_transpose(out=onat, in_=bfo)
                    rec = ep_pool.tile([128, 8], FP32, tag="rec")
                    nc.vector.reciprocal(rec, onat[:, :, D])
                    fo = ep_pool.tile([128, 8, D], FP32, tag="fo")
                    for t in range(8):
                        nc.vector.tensor_scalar_mul(
                            fo[:, t, :], onat[:, t, 0:D], rec[:, t : t + 1]
                        )
                    odst = o_d[h].rearrange("(t p) d -> p t d", p=128)
                    nc.sync.dma_start(
                        out=odst[:, 8 * hf : 8 * hf + 4, :], in_=fo[:, 0:4, :]
                    )
                    nc.sync.dma_start(
                        out=odst[:, 8 * hf + 4 : 8 * hf + 8, :], in_=fo[:, 4:8, :]
                    )
                if h + 1 < HPC:
                    vaug = vaug_next

    nc.compile()
    return nc


_NC = None


def _get_nc():
    global _NC
    if _NC is None:
        _NC = build_attention()
    return _NC


def kernel(query, key, value):
    nc = _get_nc()
    q = np.ascontiguousarray(query, dtype=np.float32).reshape(B * H, S, D)
    k = np.ascontiguousarray(key, dtype=np.float32).reshape(B * H, S, D)
    v = np.ascontiguousarray(value, dtype=np.float32).reshape(B * H, S, D)
    in_maps = [
        {
            "query": q[i * HPC : (i + 1) * HPC],
            "key": k[i * HPC : (i + 1) * HPC],
            "value": v[i * HPC : (i + 1) * HPC],
        }
        for i in range(N_CORES)
    ]
    res = run_bass_kernel_spmd(nc, in_maps, core_ids=list(range(N_CORES)))
    out = np.concatenate([res.results[i]["out"] for i in range(N_CORES)], axis=0)
    return out.reshape(B, H, S, D)


# revision 55
# speedup vs baseline: 1.1289x; 1.1289x over previous
"""Causal multi-head attention (B=2, H=16, S=2048, D=64, fp32 I/O) on 8 TRN2
NeuronCores.

Sharding: batch*heads (32 units) split 4-per-core — embarrassingly parallel,
no collectives.

Per-core kernel design (bf16 compute, fp32 PSUM accumulation):
  - scores are computed TRANSPOSED: scoresT[k, q] = K_blk @ Q^T so that the
    softmax numerators P^T[k, q] feed the P@V matmul directly as the moving
    operand (contraction dim k on partitions), with V (natural layout) as the
    stationary operand.
  - A ones-column appended to V accumulates the softmax denominator l[q] in
    the same PSUM accumulation as P@V — no separate reduction pass.
  - exp is fused with the PSUM->SBUF eviction on ScalarE (scale=1/sqrt(D)
    folded into the activation). No max-subtraction: scores ~ N(0,1), no
    overflow risk in fp32 exp.
  - Causal masking: off-diagonal blocks are skipped entirely; diagonal blocks
    get a multiplicative 0/1 upper-triangular mask after exp.
  - q is processed in halves of 1024 so PSUM fits: out^T[65,1024] (2 banks,
    double-buffered) + scoresT slots [128,1024] (2 banks, double-buffered).
  - Epilogue: copy out^T to SBUF bf16, DMA-transpose back to natural layout
    (the l column rides along), reciprocal + per-partition scale, f32 out.
  - Q^T/K^T layouts are built with bf16 DMA-transposes; K^T lands directly in
    a "paired slab" layout (kj even on partitions 0:64, kj odd on 64:128) so
    consecutive kj matmuls use disjoint PE row-groups and overlap in-array.
"""

import numpy as np

import concourse.bass as bass
import concourse.mybir as mybir
import concourse.tile as tile
from concourse.tile import add_dep_helper
from concourse import bacc
from concourse.bass_utils import run_bass_kernel_spmd
from concourse.masks import make_upper_triangular

B, H, S, D = 2, 16, 2048, 64
N_CORES = 8
HPC = (B * H) // N_CORES  # heads per core
NT = S // 128  # 16 k/q blocks of 128
FP32 = mybir.dt.float32
BF16 = mybir.dt.bfloat16


def build_attention():
    nc = bacc.Bacc("TRN2", target_bir_lowering=False)
    q_d = nc.dram_tensor("query", [HPC, S, D], FP32, kind="ExternalInput")
    k_d = nc.dram_tensor("key", [HPC, S, D], FP32, kind="ExternalInput")
    v_d = nc.dram_tensor("value", [HPC, S, D], FP32, kind="ExternalInput")
    o_d = nc.dram_tensor("out", [HPC, S, D], FP32, kind="ExternalOutput")

    with tile.TileContext(nc) as tc:
        with (
            tc.tile_pool(name="singles", bufs=1) as singles,
            tc.tile_pool(name="nat", bufs=4) as nat_pool,
            tc.tile_pool(name="bf", bufs=3) as bf_pool,
            tc.tile_pool(name="slab", bufs=2) as slab_pool,
            tc.tile_pool(name="qt", bufs=2) as qt_pool,
            tc.tile_pool(name="pt", bufs=7) as pt_pool,
            tc.tile_pool(name="ep", bufs=4) as ep_pool,
            tc.tile_pool(name="sc", bufs=2, space="PSUM") as sc_pool,
            tc.tile_pool(name="ops", bufs=2, space="PSUM") as ops_pool,
        ):
            # 0/1 mask, keep k <= q (partition = k, free = q)
            tri01 = singles.tile([128, 128], BF16, tag="tri01")
            make_upper_triangular(nc, tri01, val=1.0, diag=True)

            hm = NT // 2
            # first piece small: shortens the load->cast->transpose chain
            # in front of the very first matmul
            PIECES = ((0, 8), (8, 16))

            def load_head(h, pieces=PIECES):
                """DMA the head's q/k to SBUF (issued early; no DVE work
                so nothing blocks the DVE queue on in-flight load data).
                V is loaded separately, after the transposes: the PV
                matmuls lag a pair behind, so V is off the critical path
                and its in-flight data would otherwise delay them."""
                natQ = nat_pool.tile([128, NT, D], FP32, tag="natQ", name="natQ")
                natK = nat_pool.tile([128, NT, D], FP32, tag="natK", name="natK")
                qsrc = q_d[h].rearrange("(t p) d -> p t d", p=128)
                ksrc = k_d[h].rearrange("(t p) d -> p t d", p=128)
                for a, b in pieces:
                    nc.sync.dma_start(out=natQ[:, a:b, :], in_=qsrc[:, a:b, :])
                    nc.sync.dma_start(out=natK[:, a:b, :], in_=ksrc[:, a:b, :])
                return natQ, natK

            def load_v(h):
                natV = nat_pool.tile([128, NT, D], FP32, tag="natV", name="natV")
                vsrc = v_d[h].rearrange("(t p) d -> p t d", p=128)
                nc.sync.dma_start(out=natV[:, 0:hm, :], in_=vsrc[:, 0:hm, :])
                nc.sync.dma_start(out=natV[:, hm:NT, :], in_=vsrc[:, hm:NT, :])
                return natV

            def cast_v(natV, after=()):
                vaug = bf_pool.tile(
                    [128, NT, D + 1], BF16, tag="vaug", name="vaug"
                )
                c = nc.vector.tensor_copy(vaug[:, :, 0:D], natV)
                nc.vector.memset(vaug[:, :, D : D + 1], 1.0)
                for a in after:
                    add_dep_helper(c.ins, a.ins, sync=False,
                                   reason="v cast after epilogue copies")
                return vaug

            def cast_head(nat, after=(), pieces=PIECES):
                """bf16 casts (bfQ2 duplicates each 64-col d-block so its
                128-wide transposed chunks land Q^T_j on BOTH partition
                halves)."""
                natQ, natK = nat
                bfQ2 = bf_pool.tile(
                    [128, NT, 2, D], BF16, tag="bfQ2", name="bfQ2"
                )
                bfK = bf_pool.tile([128, NT, D], BF16, tag="bfK", name="bfK")
                casts = []
                for sl in (slice(a, b) for a, b in pieces):
                    casts.append(nc.vector.tensor_copy(bfQ2[:, sl, 0, :], natQ[:, sl, :]))
                    casts.append(nc.vector.tensor_copy(bfQ2[:, sl, 1, :], natQ[:, sl, :]))
                    casts.append(nc.vector.tensor_copy(bfK[:, sl, :], natK[:, sl, :]))
                for c in casts:
                    for a in after:
                        add_dep_helper(c.ins, a.ins, sync=False,
                                       reason="casts after epilogue copies")
                return bfQ2, bfK

            def transpose_setup(bfQ2, bfK, pieces=PIECES):
                """Blocked DMA-transposes: kslab pairs (K^T_{2j} on rows 0:64,
                K^T_{2j+1} on rows 64:128) and q-contiguous qt."""
                kslab = slab_pool.tile(
                    [128, NT // 2, 128], BF16, tag="kslab", name="kslab"
                )
                qt3 = qt_pool.tile([128, NT, 128], BF16, tag="qt", name="qt")
                bfK_f = bfK.rearrange("p t d -> p (t d)")
                bfQ2_f = bfQ2.rearrange("p t c d -> p (t c d)")
                for j0, j1 in ((a // 2, b // 2) for a, b in pieces):
                    nc.sync.dma_start_transpose(
                        out=kslab[:, j0:j1, :],
                        in_=bfK_f[:, j0 * 128 : j1 * 128],
                    )
                    nc.sync.dma_start_transpose(
                        out=qt3[:, 2 * j0 : 2 * j1, :],
                        in_=bfQ2_f[:, j0 * 256 : j1 * 256],
                    )
                return kslab, qt3.rearrange("p t i -> p (t i)")

            def cast_head_fused(nat, after=()):
                """bf16 casts into ONE combined per-half buffer
                [K-half (512) | Q2-half (1024)] so each half needs a single
                blocked DMA-transpose (fewer xbar serialization points)."""
                natQ, natK = nat
                # bfkq[:, ph, 0:512] = K tiles of half ph; [:, ph, 512:1536]
                # = duplicated Q chunks of half ph
                bfkq = bf_pool.tile(
                    [128, 2, 1536], BF16, tag="bfkq", name="bfkq"
                )
                casts = []
                for ph in range(2):
                    sl = slice(ph * hm, (ph + 1) * hm)
                    kv = bfkq[:, ph, 0:512].rearrange("p (t d) -> p t d", d=D)
                    qv = bfkq[:, ph, 512:1536].rearrange(
                        "p (t c d) -> p t c d", c=2, d=D
                    )
                    casts.append(nc.vector.tensor_copy(kv, natK[:, sl, :]))
                    casts.append(nc.vector.tensor_copy(qv[:, :, 0, :], natQ[:, sl, :]))
                    casts.append(nc.vector.tensor_copy(qv[:, :, 1, :], natQ[:, sl, :]))
                for c in casts:
                    for a in after:
                        add_dep_helper(c.ins, a.ins, sync=False,
                                       reason="casts after epilogue copies")
                return bfkq

            def transpose_setup_fused(bfkq):
                """One blocked DMA-transpose per half. Output blocks of half
                ph: 0-3 = kslab pairs 4ph..4ph+3, 4-11 = qt tiles 8ph..8ph+7
                (qt region is contiguous within each half, which is all the
                rhs slices ever span)."""
                trs = slab_pool.tile(
                    [128, 2, 12, 128], BF16, tag="trs", name="trs"
                )
                for ph in range(2):
                    nc.sync.dma_start_transpose(
                        out=trs[:, ph, :, :],
                        in_=bfkq[:, ph, :],
                    )
                trs_f = trs.rearrange("p a b c -> p (a b c)")

                def kslab_ap(rows, j):
                    return trs[rows : rows + 64, j // 4, j % 4, :]

                def qt_ap(rows, ca, cb):
                    hf = ca // 1024
                    base = hf * 1536 + 512
                    return trs_f[
                        rows : rows + 64,
                        base + ca - 1024 * hf : base + cb - 1024 * hf,
                    ]

                return kslab_ap, qt_ap

            # software-prefetch pipeline over heads: head h+1's LOADS are
            # emitted before head h's compute (the Sync engine issues its
            # stream in order), but its CASTS between the two half-sections
            # so they queue on DVE after half 0's epilogue copy (which gates
            # the single-buffered PSUM accumulator).
            staged = cast_head(load_head(0))
            nat_next = None
            natv_next = None
            vaug = None
            last_ep_copies = []
            for h in range(HPC):
                bfQ2, bfK = staged
                kslab, qt = transpose_setup(bfQ2, bfK)
                if h == 0:
                    vaug = cast_v(load_v(0))
                if h + 1 < HPC:
                    nat_next = load_head(h + 1)
                    natv_next = load_v(h + 1)

                # ---- main loop: q halves x k blocks ----
                for hf in range(2):
                    if hf == 1 and h + 1 < HPC:
                        staged = cast_head(nat_next, after=tuple(last_ep_copies))
                        vaug_next = cast_v(natv_next, after=tuple(last_ep_copies))
                    q0 = 1024 * hf  # absolute start of this q-half
                    q1 = q0 + 1024
                    kj_hi = 8 * (hf + 1)  # kj in [0, kj_hi)
                    # last kj writing each 512-bank of out^T (for stop flags)
                    last_kj = [
                        max(
                            kj
                            for kj in range(kj_hi)
                            if max(q0, 128 * kj) < q0 + 512 * (b + 1)
                        )
                        for b in range(2)
                    ]

                    outps = ops_pool.tile([80, 2, 512], FP32, tag="outps")
                    outps_f = outps.rearrange("p a b -> p (a b)")

                    # kj processed in even/odd pairs: the two QK^T matmuls use
                    # disjoint PE row-groups (partitions 0:64 vs 64:128) and
                    # run concurrently in-array. The PV matmuls are
                    # software-pipelined one pair behind: the PE queue is
                    # strictly in-order, so a PV issued right after its exp
                    # would head-of-line-block the next pair's independent
                    # QK matmuls while waiting on ScalarE.
                    def emit_pv(pair, qas, chunks):
                        # lane-outer: one V_aug weight load per kj; matmuls
                        # split on the absolute 512 grid (PSUM bank limit)
                        for lane, (kj, qa) in enumerate(zip(pair, qas)):
                            for ca, cb, ptile in chunks:
                                lo = max(ca, qa)
                                while lo < cb:
                                    hi = min(cb, q0 + 512 * ((lo - q0) // 512 + 1))
                                    b = (lo - q0) // 512
                                    nc.tensor.matmul(
                                        outps_f[0:65, lo - q0 : hi - q0],
                                        vaug[:, kj, :],
                                        ptile[:, lane, lo - ca : hi - ca],
                                        start=(kj == 0),
                                        stop=(kj == last_kj[b]),
                                    )
                                    lo = hi

                    pending = []
                    for pj in range(kj_hi // 2):
                        pair = (2 * pj, 2 * pj + 1)
                        qas = [max(q0, 128 * kj) for kj in pair]
                        # Both lanes of a pair share one PSUM slot and one exp:
                        # the two QK^T matmuls then become ready together (same
                        # WAR release) and execute concurrently in disjoint PE
                        # row-groups.
                        chunks = []
                        for ca in range(qas[0], q1, 512):
                            cb = min(ca + 512, q1)
                            cols = cb - ca
                            slot = sc_pool.tile(
                                [128, 2, 512], FP32, tag="slot", name="slot"
                            )
                            for lane, (kj, qa) in enumerate(zip(pair, qas)):
                                lo = max(ca, qa)
                                if lo >= cb:
                                    continue
                                rows = (kj % 2) * 64
                                nc.tensor.matmul(
                                    slot[:, lane, lo - ca : cols],
                                    kslab[rows : rows + 64, kj // 2, :],
                                    qt[rows : rows + 64, lo:cb],
                                    start=True,
                                    stop=True,
                                )
                            ptile = pt_pool.tile(
                                [128, 2, 512], BF16, tag="ptile", name="ptile"
                            )
                            # the odd lane's first 128 cols in its diagonal
                            # chunk exp stale PSUM; PV never reads them
                            nc.scalar.activation(
                                ptile[:, :, 0:cols],
                                slot[:, :, 0:cols],
                                mybir.ActivationFunctionType.Exp,
                                scale=1.0 / np.sqrt(D),
                            )
                            for lane, (kj, qa) in enumerate(zip(pair, qas)):
                                dg = 128 * kj
                                if ca <= dg < cb:  # diagonal block in chunk
                                    # on gpsimd: keeps the mask off the DVE
                                    # queue, which cast/epilogue work clogs
                                    # at head boundaries
                                    nc.gpsimd.tensor_mul(
                                        ptile[:, lane, dg - ca : dg - ca + 128],
                                        ptile[:, lane, dg - ca : dg - ca + 128],
                                        tri01,
                                    )
                            chunks.append((ca, cb, ptile))
                        pending.append((pair, qas, chunks))
                        if pj >= 1:
                            emit_pv(*pending.pop(0))
                    for args in pending:
                        emit_pv(*args)

                    # ---- epilogue for this (head, half) ----
                    # rows 65:80 copy PSUM garbage; they transpose into
                    # columns 65:80 of onat which are never read
                    bfo = ep_pool.tile([80, 1024], BF16, tag="bfo")
                    # per-bank copies: each releases its PSUM bank (WAR) as
                    # soon as it finishes, unblocking the next half's PV
                    epc1 = nc.vector.tensor_copy(bfo[:, 0:512], outps_f[0:80, 0:512])
                    epc2 = nc.vector.tensor_copy(bfo[:, 512:1024], outps_f[0:80, 512:1024])
                    last_ep_copies[:] = [epc1, epc2]
                    onat = ep_pool.tile([128, 8, 80], BF16, tag="onat")
                    nc.sync.dma_start_transpose(out=onat, in_=bfo)
                    rec = ep_pool.tile([128, 8], FP32, tag="rec")
                    nc.vector.reciprocal(rec, onat[:, :, D])
                    fo = ep_pool.tile([128, 8, D], FP32, tag="fo")
                    for t in range(8):
                        nc.vector.tensor_scalar_mul(
                            fo[:, t, :], onat[:, t, 0:D], rec[:, t : t + 1]
                        )
                    odst = o_d[h].rearrange("(t p) d -> p t d", p=128)
                    nc.sync.dma_start(
                        out=odst[:, 8 * hf : 8 * hf + 4, :], in_=fo[:, 0:4, :]
                    )
                    nc.sync.dma_start(
                        out=odst[:, 8 * hf + 4 : 8 * hf + 8, :], in_=fo[:, 4:8, :]
                    )
                if h + 1 < HPC:
                    vaug = vaug_next

    nc.compile()
    return nc


_NC = None


def _get_nc():
    global _NC
    if _NC is None:
        _NC = build_attention()
    return _NC


def kernel(query, key, value):
    nc = _get_nc()
    q = np.ascontiguousarray(query, dtype=np.float32).reshape(B * H, S, D)
    k = np.ascontiguousarray(key, dtype=np.float32).reshape(B * H, S, D)
    v = np.ascontiguousarray(value, dtype=np.float32).reshape(B * H, S, D)
    in_maps = [
        {
            "query": q[i * HPC : (i + 1) * HPC],
            "key": k[i * HPC : (i + 1) * HPC],
            "value": v[i * HPC : (i + 1) * HPC],
        }
        for i in range(N_CORES)
    ]
    res = run_bass_kernel_spmd(nc, in_maps, core_ids=list(range(N_CORES)))
    out = np.concatenate([res.results[i]["out"] for i in range(N_CORES)], axis=0)
    return out.reshape(B, H, S, D)
